# revision 5
# baseline (speedup 1.0000x reference)
"""Trainium2 Bass kernel for nn_CorrKernel (SpatialCorrelationSampler).

corr[b, p, y, x] = sum_c f0[b,c,y,x] * f1[b,c,y+dy,x+dx],
(dy,dx) in [-4,4]^2 -> p = (dy+4)*9 + (dx+4); OOB -> 0.

Strategy (8 cores = 4 batches x 2 y-halves of 48 rows):
  - Host passes per-core shards transposed to (c, x, y) layout; f1 is
    zero-padded by the +-4 halo in both spatial dims.
  - Per image column x, one TensorE matmul pair (K=256 via 2 accumulated
    128-chunks, float32r): lhsT = f0 strip (128c x 48y), rhs = f1 window
    (128c x [56 y' x 9 dx] = 504 cols). In PSUM, pixel (y) then owns the
    contiguous 81-value run at cols [9y, 9y+81) in reference p-order.
  - Evacuate PSUM->SBUF (DVE/ACT), then one DMA per (y, x-chunk) writes the
    pixel-major (y, x, 81) device output. Host transposes to (81, H, W).
"""

import sys

for _p in ("/opt/trn_rl_repo", "/root/.axon_site", "/root/.axon_site/_ro/trn_rl_repo"):
    if _p not in sys.path:
        sys.path.append(_p)

import ml_dtypes
import numpy as np
import concourse.bass as bass
import concourse.mybir as mybir
import concourse.tile as tile
from concourse.bass_utils import run_bass_kernel_spmd

B, C, H, W = 4, 256, 96, 160
D = 4               # max displacement
P = 2 * D + 1       # 9
P2 = P * P          # 81
HY = H // 2         # 48 rows per core
WP = W + 2 * D      # 168 padded x
HP = HY + 2 * D     # 56 padded y
N_CORES = 8
X_STRIPES = 4       # input slabs split along x for load/compute overlap
SW = W // X_STRIPES  # 40
CHUNK = 8           # strips per score chunk (output granularity)
NCOLS = HP * P      # 504 matmul free dim (one PSUM bank)

USE_F32R = False
USE_BF16 = True


def _split_ctrl_waits(nc):
    """This walrus build allows only ONE sync-wait per instruction;
    spill extra waits onto dedicated single-wait NoOps just before it."""
    for f in nc.m.functions:
        for blk in f.blocks:
            new_insts = []
            for inst in blk.instructions:
                si = inst.sync_info
                if (
                    si is not None
                    and si.on_wait
                    and len(si.on_wait) > 1
                ):
                    waits = list(si.on_wait)
                    for w in waits[:-1]:
                        nop = mybir.InstNoOp(
                            name=nc.get_next_instruction_name(), ins=[], outs=[]
                        )
                        nop.engine = inst.engine
                        nop.sync_info = mybir.SyncInfo(on_wait=[w], on_update=[])
                        new_insts.append(nop)
                    si.on_wait = [waits[-1]]
                new_insts.append(inst)
            blk.instructions[:] = new_insts


def _build_nc():
    nc = bass.Bass()
    mm_dt = (mybir.dt.bfloat16 if USE_BF16 else
             (mybir.dt.float32r if USE_F32R else mybir.dt.float32))
    f0 = nc.dram_tensor("f0", [C, W, HY], mm_dt, kind="ExternalInput")
    f1 = nc.dram_tensor("f1", [C, WP, HP], mm_dt, kind="ExternalInput")
    out = nc.dram_tensor("out", [HY, W, P2], mybir.dt.float32, kind="ExternalOutput")

    with tile.TileContext(nc) as tc:
        with tc.tile_pool(name="f0pool", bufs=1) as f0p, \
             tc.tile_pool(name="f1pool", bufs=1) as f1p, \
             tc.tile_pool(name="score", bufs=2) as scp, \
             tc.tile_pool(name="psum", bufs=8, space="PSUM") as psp:
            f0t = {}
            f1t = {}
            for h in range(2):
                cs = slice(128 * h, 128 * (h + 1))
                for s in range(X_STRIPES):
                    t0 = f0p.tile([128, SW, HY], mm_dt, tag=f"f0_{h}_{s}")
                    nc.sync.dma_start(t0[:], f0[cs, s * SW:(s + 1) * SW, :])
                    f0t[h, s] = t0
                    t1 = f1p.tile([128, SW + 2 * D, HP], mm_dt,
                                  tag=f"f1_{h}_{s}")
                    nc.sync.dma_start(t1[:], f1[cs, s * SW:s * SW + SW + 2 * D, :])
                    f1t[h, s] = t1

            with nc.allow_non_contiguous_dma(reason="skewed band extraction"):
                for xc in range(0, W, CHUNK):
                    sc = scp.tile([HY, CHUNK, NCOLS], mybir.dt.float32, tag="score")
                    for xl in range(CHUNK):
                        x = xc + xl
                        s = x // SW
                        xo = x - s * SW
                        ps = psp.tile([HY, NCOLS], mybir.dt.float32, tag="ps")
                        for h in range(2):
                            lhsT = f0t[h, s][:, xo, :]
                            rhs = (
                                f1t[h, s][:, xo:xo + P, :]
                                .rearrange("c x y -> c y x")
                            )
                            nc.tensor.matmul(
                                ps[:], lhsT, rhs, start=(h == 0), stop=(h == 1)
                            )
                        if x % 3 == 2:
                            nc.scalar.copy(out=sc[:, xl, :], in_=ps[:])
                        else:
                            nc.vector.tensor_copy(out=sc[:, xl, :], in_=ps[:])
                    for Y in range(HY):
                        src = sc[Y:Y + 1, :, 9 * Y: 9 * Y + P2]
                        dst = out[Y:Y + 1, xc:xc + CHUNK, :]
                        eng = nc.sync if (Y % 2 == 0) else nc.scalar
                        eng.dma_start(dst, src)

    _split_ctrl_waits(nc)
    return nc


_NC = None


def _get_nc():
    global _NC
    if _NC is None:
        _NC = _build_nc()
    return _NC


def _shard_inputs(fmap0, fmap1):
    fmap0 = np.ascontiguousarray(np.asarray(fmap0, dtype=np.float32))
    fmap1 = np.ascontiguousarray(np.asarray(fmap1, dtype=np.float32))
    in_maps = []
    for core in range(N_CORES):
        b, half = divmod(core, 2)
        y0 = half * HY
        f0s = np.transpose(fmap0[b, :, y0:y0 + HY, :], (0, 2, 1))  # (C, W, HY)
        f1pad = np.zeros((C, WP, HP), dtype=np.float32)
        ylo, yhi = y0 - D, y0 + HY + D
        slo, shi = max(ylo, 0), min(yhi, H)
        f1s = np.transpose(fmap1[b, :, slo:shi, :], (0, 2, 1))  # (C, W, ny)
        f1pad[:, D:D + W, slo - ylo: slo - ylo + (shi - slo)] = f1s
        cast = ml_dtypes.bfloat16 if USE_BF16 else np.float32
        in_maps.append({
            "f0": np.ascontiguousarray(f0s).astype(cast),
            "f1": np.ascontiguousarray(f1pad).astype(cast),
        })
    return in_maps


def _gather(results):
    out = np.empty((B, P2, H, W), dtype=np.float32)
    for core in range(N_CORES):
        b, half = divmod(core, 2)
        y0 = half * HY
        dev = results[core]["out"]  # (HY, W, P2)
        out[b, :, y0:y0 + HY, :] = np.transpose(dev, (2, 0, 1))
    return out


def kernel(fmap0, fmap1):
    nc = _get_nc()
    in_maps = _shard_inputs(fmap0, fmap1)
    res = run_bass_kernel_spmd(nc, in_maps, core_ids=list(range(N_CORES)))
    return _gather(res.results)


# used by test.py for profiling without rebuilding
def run_traced(fmap0, fmap1):
    nc = _get_nc()
    in_maps = _shard_inputs(fmap0, fmap1)
    res = run_bass_kernel_spmd(
        nc, in_maps, core_ids=list(range(N_CORES)), trace=True
    )
    return _gather(res.results), res


# revision 6
# speedup vs baseline: 1.4026x; 1.4026x over previous
"""Trainium2 Bass kernel for nn_CorrKernel (SpatialCorrelationSampler).

corr[b, p, y, x] = sum_c f0[b,c,y,x] * f1[b,c,y+dy,x+dx],
(dy,dx) in [-4,4]^2 -> p = (dy+4)*9 + (dx+4); OOB -> 0.

Strategy (8 cores = 4 batches x 2 y-halves of 48 rows):
  - Host passes per-core shards transposed to (c, x, y) layout; f1 is
    zero-padded by the +-4 halo in both spatial dims.
  - Per image column x, one TensorE matmul pair (K=256 via 2 accumulated
    128-chunks, float32r): lhsT = f0 strip (128c x 48y), rhs = f1 window
    (128c x [56 y' x 9 dx] = 504 cols). In PSUM, pixel (y) then owns the
    contiguous 81-value run at cols [9y, 9y+81) in reference p-order.
  - Evacuate PSUM->SBUF (DVE/ACT), then one DMA per (y, x-chunk) writes the
    pixel-major (y, x, 81) device output. Host transposes to (81, H, W).
"""

import sys

for _p in ("/opt/trn_rl_repo", "/root/.axon_site", "/root/.axon_site/_ro/trn_rl_repo"):
    if _p not in sys.path:
        sys.path.append(_p)

import ml_dtypes
import numpy as np
import concourse.bass as bass
import concourse.mybir as mybir
import concourse.tile as tile
from concourse.bass_utils import run_bass_kernel_spmd

B, C, H, W = 4, 256, 96, 160
D = 4               # max displacement
P = 2 * D + 1       # 9
P2 = P * P          # 81
HY = H // 2         # 48 rows per core
WP = W + 2 * D      # 168 padded x
HP = HY + 2 * D     # 56 padded y
N_CORES = 8
X_STRIPES = 4       # input slabs split along x for load/compute overlap
SW = W // X_STRIPES  # 40
CHUNK = 8           # strips per score chunk (output granularity)
NCOLS = HP * P      # 504 matmul free dim (one PSUM bank)

USE_F32R = False
USE_BF16 = True


def _split_ctrl_waits(nc):
    """This walrus build allows only ONE sync-wait per instruction;
    spill extra waits onto dedicated single-wait NoOps just before it."""
    for f in nc.m.functions:
        for blk in f.blocks:
            new_insts = []
            for inst in blk.instructions:
                si = inst.sync_info
                if (
                    si is not None
                    and si.on_wait
                    and len(si.on_wait) > 1
                ):
                    waits = list(si.on_wait)
                    for w in waits[:-1]:
                        nop = mybir.InstNoOp(
                            name=nc.get_next_instruction_name(), ins=[], outs=[]
                        )
                        nop.engine = inst.engine
                        nop.sync_info = mybir.SyncInfo(on_wait=[w], on_update=[])
                        new_insts.append(nop)
                    si.on_wait = [waits[-1]]
                new_insts.append(inst)
            blk.instructions[:] = new_insts


def _build_nc():
    nc = bass.Bass()
    mm_dt = (mybir.dt.bfloat16 if USE_BF16 else
             (mybir.dt.float32r if USE_F32R else mybir.dt.float32))
    f0 = nc.dram_tensor("f0", [C, W, HY], mm_dt, kind="ExternalInput")
    f1 = nc.dram_tensor("f1", [C, WP, HP], mm_dt, kind="ExternalInput")
    out = nc.dram_tensor("out", [HY, W, P2], mybir.dt.float32, kind="ExternalOutput")

    with tile.TileContext(nc) as tc:
        with tc.tile_pool(name="f0pool", bufs=1) as f0p, \
             tc.tile_pool(name="f1pool", bufs=1) as f1p, \
             tc.tile_pool(name="score", bufs=2) as scp, \
             tc.tile_pool(name="psum", bufs=8, space="PSUM") as psp:
            f0t = {}
            f1t = {}
            for h in range(2):
                cs = slice(128 * h, 128 * (h + 1))
                for s in range(X_STRIPES):
                    t0 = f0p.tile([128, SW, HY], mm_dt, tag=f"f0_{h}_{s}")
                    nc.sync.dma_start(t0[:], f0[cs, s * SW:(s + 1) * SW, :])
                    f0t[h, s] = t0
                    t1 = f1p.tile([128, SW + 2 * D, HP], mm_dt,
                                  tag=f"f1_{h}_{s}")
                    nc.sync.dma_start(t1[:], f1[cs, s * SW:s * SW + SW + 2 * D, :])
                    f1t[h, s] = t1

            with nc.allow_non_contiguous_dma(reason="skewed band extraction"):
                for xc in range(0, W, CHUNK):
                    sc = scp.tile([HY, CHUNK, NCOLS], mybir.dt.float32, tag="score")
                    for xl in range(CHUNK):
                        x = xc + xl
                        s = x // SW
                        xo = x - s * SW
                        ps = psp.tile([HY, NCOLS], mybir.dt.float32, tag="ps")
                        for h in range(2):
                            lhsT = f0t[h, s][:, xo, :]
                            rhs = (
                                f1t[h, s][:, xo:xo + P, :]
                                .rearrange("c x y -> c y x")
                            )
                            nc.tensor.matmul(
                                ps[:], lhsT, rhs, start=(h == 0), stop=(h == 1)
                            )
                        if x % 3 == 2:
                            nc.scalar.copy(out=sc[:, xl, :], in_=ps[:])
                        else:
                            nc.vector.tensor_copy(out=sc[:, xl, :], in_=ps[:])
                    # one DMA per chunk: the per-pixel skew (row Y's 81-value
                    # run starts at col 9Y) folds into a single AP dim whose
                    # step crosses one partition plus 9 elements.
                    row_elems = CHUNK * NCOLS
                    src = bass.AP(
                        sc.tensor,
                        sc.offset,
                        [[row_elems + 9, HY], [NCOLS, CHUNK], [1, P2]],
                    )
                    dst = out[:, xc:xc + CHUNK, :]
                    eng = nc.sync if (xc // CHUNK) % 2 == 0 else nc.scalar
                    eng.dma_start(dst, src)

    _split_ctrl_waits(nc)
    return nc


_NC = None


def _get_nc():
    global _NC
    if _NC is None:
        _NC = _build_nc()
    return _NC


def _shard_inputs(fmap0, fmap1):
    fmap0 = np.ascontiguousarray(np.asarray(fmap0, dtype=np.float32))
    fmap1 = np.ascontiguousarray(np.asarray(fmap1, dtype=np.float32))
    in_maps = []
    for core in range(N_CORES):
        b, half = divmod(core, 2)
        y0 = half * HY
        f0s = np.transpose(fmap0[b, :, y0:y0 + HY, :], (0, 2, 1))  # (C, W, HY)
        f1pad = np.zeros((C, WP, HP), dtype=np.float32)
        ylo, yhi = y0 - D, y0 + HY + D
        slo, shi = max(ylo, 0), min(yhi, H)
        f1s = np.transpose(fmap1[b, :, slo:shi, :], (0, 2, 1))  # (C, W, ny)
        f1pad[:, D:D + W, slo - ylo: slo - ylo + (shi - slo)] = f1s
        cast = ml_dtypes.bfloat16 if USE_BF16 else np.float32
        in_maps.append({
            "f0": np.ascontiguousarray(f0s).astype(cast),
            "f1": np.ascontiguousarray(f1pad).astype(cast),
        })
    return in_maps


def _gather(results):
    out = np.empty((B, P2, H, W), dtype=np.float32)
    for core in range(N_CORES):
        b, half = divmod(core, 2)
        y0 = half * HY
        dev = results[core]["out"]  # (HY, W, P2)
        out[b, :, y0:y0 + HY, :] = np.transpose(dev, (2, 0, 1))
    return out


def kernel(fmap0, fmap1):
    nc = _get_nc()
    in_maps = _shard_inputs(fmap0, fmap1)
    res = run_bass_kernel_spmd(nc, in_maps, core_ids=list(range(N_CORES)))
    return _gather(res.results)


# used by test.py for profiling without rebuilding
def run_traced(fmap0, fmap1):
    nc = _get_nc()
    in_maps = _shard_inputs(fmap0, fmap1)
    res = run_bass_kernel_spmd(
        nc, in_maps, core_ids=list(range(N_CORES)), trace=True
    )
    return _gather(res.results), res


# revision 7
# speedup vs baseline: 4.0621x; 2.8961x over previous
"""Trainium2 Bass kernel for nn_CorrKernel (SpatialCorrelationSampler).

corr[b, p, y, x] = sum_c f0[b,c,y,x] * f1[b,c,y+dy,x+dx],
(dy,dx) in [-4,4]^2 -> p = (dy+4)*9 + (dx+4); OOB -> 0.

Strategy (8 cores = 4 batches x 2 y-halves of 48 rows):
  - Host passes per-core shards transposed to (c, x, y) layout; f1 is
    zero-padded by the +-4 halo in both spatial dims.
  - Per image column x, one TensorE matmul pair (K=256 via 2 accumulated
    128-chunks, float32r): lhsT = f0 strip (128c x 48y), rhs = f1 window
    (128c x [56 y' x 9 dx] = 504 cols). In PSUM, pixel (y) then owns the
    contiguous 81-value run at cols [9y, 9y+81) in reference p-order.
  - Evacuate PSUM->SBUF (DVE/ACT), then one DMA per (y, x-chunk) writes the
    pixel-major (y, x, 81) device output. Host transposes to (81, H, W).
"""

import sys

for _p in ("/opt/trn_rl_repo", "/root/.axon_site", "/root/.axon_site/_ro/trn_rl_repo"):
    if _p not in sys.path:
        sys.path.append(_p)

import ml_dtypes
import numpy as np
import concourse.bass as bass
import concourse.mybir as mybir
import concourse.tile as tile
from concourse.bass_utils import run_bass_kernel_spmd

B, C, H, W = 4, 256, 96, 160
D = 4               # max displacement
P = 2 * D + 1       # 9
P2 = P * P          # 81
HY = H // 2         # 48 rows per core
WP = W + 2 * D      # 168 padded x
HP = HY + 2 * D     # 56 padded y
N_CORES = 8
X_STRIPES = 4       # input slabs split along x for load/compute overlap
SW = W // X_STRIPES  # 40
CHUNK = 8           # strips per score chunk (output granularity)
NCOLS = HP * P      # 504 matmul free dim (one PSUM bank)

USE_F32R = False
USE_BF16 = True


def _split_ctrl_waits(nc):
    """This walrus build allows only ONE sync-wait per instruction;
    spill extra waits onto dedicated single-wait NoOps just before it."""
    for f in nc.m.functions:
        for blk in f.blocks:
            new_insts = []
            for inst in blk.instructions:
                si = inst.sync_info
                if (
                    si is not None
                    and si.on_wait
                    and len(si.on_wait) > 1
                ):
                    waits = list(si.on_wait)
                    for w in waits[:-1]:
                        nop = mybir.InstNoOp(
                            name=nc.get_next_instruction_name(), ins=[], outs=[]
                        )
                        nop.engine = inst.engine
                        nop.sync_info = mybir.SyncInfo(on_wait=[w], on_update=[])
                        new_insts.append(nop)
                    si.on_wait = [waits[-1]]
                new_insts.append(inst)
            blk.instructions[:] = new_insts


def _build_nc():
    nc = bass.Bass()
    mm_dt = (mybir.dt.bfloat16 if USE_BF16 else
             (mybir.dt.float32r if USE_F32R else mybir.dt.float32))
    f0 = nc.dram_tensor("f0", [C, W, HY], mm_dt, kind="ExternalInput")
    f1 = nc.dram_tensor("f1", [C, WP, HP], mm_dt, kind="ExternalInput")
    out = nc.dram_tensor("out", [HY, W, P2], mybir.dt.float32, kind="ExternalOutput")

    with tile.TileContext(nc) as tc:
        with tc.tile_pool(name="f0pool", bufs=1) as f0p, \
             tc.tile_pool(name="f1pool", bufs=1) as f1p, \
             tc.tile_pool(name="score", bufs=2) as scp, \
             tc.tile_pool(name="psum", bufs=8, space="PSUM") as psp:
            f0t = {}
            f1t = {}
            for h in range(2):
                cs = slice(128 * h, 128 * (h + 1))
                for s in range(X_STRIPES):
                    t0 = f0p.tile([128, SW, HY], mm_dt, tag=f"f0_{h}_{s}")
                    nc.sync.dma_start(t0[:], f0[cs, s * SW:(s + 1) * SW, :])
                    f0t[h, s] = t0
                    t1 = f1p.tile([128, SW + 2 * D, HP], mm_dt,
                                  tag=f"f1_{h}_{s}")
                    nc.sync.dma_start(t1[:], f1[cs, s * SW:s * SW + SW + 2 * D, :])
                    f1t[h, s] = t1

            with nc.allow_non_contiguous_dma(reason="skewed band extraction"):
                for xc in range(0, W, CHUNK):
                    sc = scp.tile([HY, CHUNK, HP, P], mybir.dt.float32, tag="score")
                    for xl in range(CHUNK):
                        x = xc + xl
                        s = x // SW
                        xo = x - s * SW
                        ps = psp.tile([HY, P, HP], mybir.dt.float32, tag="ps")
                        for h in range(2):
                            lhsT = f0t[h, s][:, xo, :]
                            # native (dx-outer, y-contiguous) streaming: 9
                            # contiguous 56-col segments, no per-col overhead
                            rhs = f1t[h, s][:, xo:xo + P, :]
                            nc.tensor.matmul(
                                ps[:], lhsT, rhs, start=(h == 0), stop=(h == 1)
                            )
                        # evac permutes (dx, y') -> (y', dx) so each pixel's
                        # 81 outputs stay contiguous for the extraction DMA
                        src_ap = ps[:].rearrange("m dx y -> m y dx")
                        if x % 3 == 2:
                            nc.scalar.copy(out=sc[:, xl], in_=src_ap)
                        else:
                            nc.vector.tensor_copy(out=sc[:, xl], in_=src_ap)
                    # one DMA per chunk: the per-pixel skew (row Y's 81-value
                    # run starts at col 9Y) folds into a single AP dim whose
                    # step crosses one partition plus 9 elements.
                    row_elems = CHUNK * NCOLS
                    src = bass.AP(
                        sc.tensor,
                        sc.offset,
                        [[row_elems + 9, HY], [NCOLS, CHUNK], [1, P2]],
                    )
                    dst = out[:, xc:xc + CHUNK, :]
                    eng = nc.sync if (xc // CHUNK) % 2 == 0 else nc.scalar
                    eng.dma_start(dst, src)

    _split_ctrl_waits(nc)
    return nc


_NC = None


def _get_nc():
    global _NC
    if _NC is None:
        _NC = _build_nc()
    return _NC


def _shard_inputs(fmap0, fmap1):
    fmap0 = np.ascontiguousarray(np.asarray(fmap0, dtype=np.float32))
    fmap1 = np.ascontiguousarray(np.asarray(fmap1, dtype=np.float32))
    in_maps = []
    for core in range(N_CORES):
        b, half = divmod(core, 2)
        y0 = half * HY
        f0s = np.transpose(fmap0[b, :, y0:y0 + HY, :], (0, 2, 1))  # (C, W, HY)
        f1pad = np.zeros((C, WP, HP), dtype=np.float32)
        ylo, yhi = y0 - D, y0 + HY + D
        slo, shi = max(ylo, 0), min(yhi, H)
        f1s = np.transpose(fmap1[b, :, slo:shi, :], (0, 2, 1))  # (C, W, ny)
        f1pad[:, D:D + W, slo - ylo: slo - ylo + (shi - slo)] = f1s
        cast = ml_dtypes.bfloat16 if USE_BF16 else np.float32
        in_maps.append({
            "f0": np.ascontiguousarray(f0s).astype(cast),
            "f1": np.ascontiguousarray(f1pad).astype(cast),
        })
    return in_maps


def _gather(results):
    out = np.empty((B, P2, H, W), dtype=np.float32)
    for core in range(N_CORES):
        b, half = divmod(core, 2)
        y0 = half * HY
        dev = results[core]["out"]  # (HY, W, P2)
        out[b, :, y0:y0 + HY, :] = np.transpose(dev, (2, 0, 1))
    return out


def kernel(fmap0, fmap1):
    nc = _get_nc()
    in_maps = _shard_inputs(fmap0, fmap1)
    res = run_bass_kernel_spmd(nc, in_maps, core_ids=list(range(N_CORES)))
    return _gather(res.results)


# used by test.py for profiling without rebuilding
def run_traced(fmap0, fmap1):
    nc = _get_nc()
    in_maps = _shard_inputs(fmap0, fmap1)
    res = run_bass_kernel_spmd(
        nc, in_maps, core_ids=list(range(N_CORES)), trace=True
    )
    return _gather(res.results), res
